# revision 20
# baseline (speedup 1.0000x reference)
"""Distributed Trainium2 (Bass/Tile) kernel for AdaptiveGCNLayer.

Reference semantics (N=4096 nodes, C=512 channels):
    adj   = x @ W_adj @ x.T + I                      [N, N]
    adj   = d^-1/2 * adj * d^-1/2   (row sums d)     -- values then DISCARDED:
    A     = (adj != 0) with forced unit diagonal     (dense_to_sparse keeps only
                                                      the nonzero pattern)
    deg   = A.sum(1); dis = deg^-1/2 (0 if deg<=0)
    out   = (dis[:,None] * A * dis[None,:]) @ (x @ W_gcn) + b

Scaling rows/cols by nonzero (or NaN/inf) factors never changes the !=0
pattern, so A == (x @ W_adj @ x.T != 0) except on the measure-zero event of
an exactly-zero f32 entry; the first normalization is therefore not
materialized, and the adjacency can be computed at any precision (fp8 here)
since only its zero pattern survives.  deg >= 1 always (forced diagonal).

Sharding (8 cores, 1-D node partition, R=512 rows each): core i computes its
adjacency block in TRANSPOSED layout adjT [N, R] (directly usable as the
stationary operand of the final aggregation), masks it to {0,1} bf16,
reduces mask -> deg for its rows (ones-matmul on the TensorEngine), and
exchanges [xg_shard ; deg_shard] in a single AllGather (deg rides as two
bitcast rows of the payload).  The gathered xg is scaled by dis and
aggregated: out_rows = dis_r * (A_rows @ (dis * xg)) + b, bf16 matmuls with
fp32 PSUM accumulation.

Overlap / latency structure (this environment has ~40us rank-dispatch skew
and ~11us collective-stream start latency):
  - a tiny bias AllReduce triggered first thing absorbs the skew barrier
    while local compute proceeds
  - the single payload AllGather triggers right at the end of the adjacency
    phase; everything before it runs under the skew window
  - adjacency matmuls run fp8e4m3 DoubleRow (zero-pattern precision only)
  - the final aggregation is m-outer so each PSUM bank accumulates a long
    33-matmul chain (bank-cycling triggers the HAM oscillation mode)
"""

import numpy as np

from concourse import bacc, mybir, tile
from concourse.bass_utils import run_bass_kernel_spmd

N_CORES = 8
N = 4096               # nodes
C = 512                # channels (C_IN == C_OUT)
R = N // N_CORES       # 512 rows per core
P = 128                # SBUF partitions
KT = C // P            # 4 contraction tiles
NT = N // P            # 32 node tiles
MT = R // P            # 4 row tiles per core
BR = R + 2             # payload rows per rank: xg rows + 2 bitcast deg rows

F32 = mybir.dt.float32
BF16 = mybir.dt.bfloat16
F8 = mybir.dt.float8e4
BF = mybir.dt.np(BF16)
F8NP = mybir.dt.np(F8)
DR = mybir.MatmulPerfMode.DoubleRow

_cache = {}


def _build():
    nc = bacc.Bacc("TRN2", target_bir_lowering=False, debug=False,
                   num_devices=N_CORES)

    xT8 = nc.dram_tensor("xT8", [C, N], F8, kind="ExternalInput")      # x^T, full
    xTs8 = nc.dram_tensor("xTs8", [C, R], F8, kind="ExternalInput")    # own cols
    adjW8 = nc.dram_tensor("adjW8", [C, C], F8, kind="ExternalInput")
    xTs = nc.dram_tensor("xTs", [C, R], BF16, kind="ExternalInput")
    gcnW = nc.dram_tensor("gcnW", [C, C], BF16, kind="ExternalInput")
    bias = nc.dram_tensor("bias", [1, C], BF16, kind="ExternalInput")
    out = nc.dram_tensor("out", [R, C], F32, kind="ExternalOutput")

    rg = [list(range(N_CORES))]

    with tile.TileContext(nc) as tc:
        with (
            tc.tile_pool(name="sb", bufs=1) as sb,
            tc.tile_pool(name="sbo", bufs=2) as sbo,
            tc.tile_pool(name="dram", bufs=1, space="DRAM") as dram,
            tc.tile_pool(name="ps_a", bufs=1, space="PSUM") as ps_a,
            tc.tile_pool(name="ps_adj", bufs=2, space="PSUM") as ps_adj,
            tc.tile_pool(name="ps_deg", bufs=1, space="PSUM") as ps_deg,
            tc.tile_pool(name="ps_fin", bufs=2, space="PSUM") as ps_fin,
        ):
            # ---- skew absorber: tiny AllReduce(max) on the bias ------------
            # Bias values are identical on every rank, so max() is the
            # identity; this collective exists to pay the rank-skew entry
            # barrier while the compute below proceeds.
            bias_bin = dram.tile([1, C], BF16, name="bias_bin", tag="bias_bin")
            bias_bout = dram.tile([1, C], BF16, addr_space="Shared",
                                  name="bias_bout", tag="bias_bout")
            nc.gpsimd.dma_start(bias_bin[:, :], bias[:, :])
            nc.gpsimd.collective_compute(
                "AllReduce", mybir.AluOpType.max, replica_groups=rg,
                ins=[bias_bin.opt()], outs=[bias_bout.opt()])
            bias_sb = sb.tile([1, C], BF16, name="bias_sb", tag="bias_sb")
            nc.sync.dma_start(bias_sb[:, :], bias_bout[:, :])

            # ---- input loads ------------------------------------------------
            xTs_sb = [sb.tile([P, R], BF16, name=f"xTs{k}", tag=f"xTs{k}") for k in range(KT)]
            gcnW_sb = [sb.tile([P, C], BF16, name=f"gcnW{k}", tag=f"gcnW{k}") for k in range(KT)]
            # fp8 operands in DoubleRow layout [P, k-subtile, free]
            adjW8_sb = sb.tile([P, KT, C], F8, name="adjW8_sb", tag="adjW8_sb")
            xTs8_sb = sb.tile([P, KT, R], F8, name="xTs8_sb", tag="xTs8_sb")
            xT8_sb = sb.tile([P, KT, N], F8, name="xT8_sb", tag="xT8_sb")
            ones_col = sb.tile([P, 1], BF16, name="ones_col", tag="ones_col")
            scr = sb.tile([1, 8], F32, name="scr", tag="scr")

            for k in range(KT):
                nc.sync.dma_start(xTs_sb[k][:, :], xTs[P * k:P * (k + 1), :])
                nc.sync.dma_start(gcnW_sb[k][:, :], gcnW[P * k:P * (k + 1), :])
            for k in range(KT):
                nc.sync.dma_start(adjW8_sb[:, k, :], adjW8[P * k:P * (k + 1), :])
                nc.sync.dma_start(xTs8_sb[:, k, :], xTs8[P * k:P * (k + 1), :])
            for k in range(KT):
                nc.sync.dma_start(xT8_sb[:, k, :], xT8[P * k:P * (k + 1), :])
            nc.vector.memset(ones_col[:, :], 1.0)
            # preload the DVE reciprocal / ACT sqrt lookup tables off the
            # critical path (first use otherwise costs ~1.3us each)
            nc.vector.memset(scr[:, 0:4], 4.0)
            nc.vector.reciprocal(scr[:, 4:8], scr[:, 0:4])
            nc.scalar.sqrt(scr[:, 4:8], scr[:, 0:4])

            # ---- phase 1b: xg[r, f] = sum_c x[r, c] W_gcn[c, f] (own rows) --
            yb_in = dram.tile([BR, C], BF16, name="yb_in", tag="yb_in")
            yb_out = dram.tile([N_CORES * BR, C], BF16, addr_space="Shared",
                               name="yb_out", tag="yb_out")
            xg_sb = [sb.tile([P, C], BF16, name=f"xg{m}", tag=f"xg{m}") for m in range(MT)]
            for m in range(MT):
                pa = ps_a.tile([P, C], F32, name=f"psg{m}", tag="psa")
                for k in range(KT):
                    nc.tensor.matmul(pa[:, :],
                                     xTs_sb[k][:, P * m:P * (m + 1)],
                                     gcnW_sb[k][:, :],
                                     start=(k == 0), stop=(k == KT - 1))
                nc.vector.tensor_copy(xg_sb[m][:, :], pa[:, :])
                nc.gpsimd.dma_start(yb_in[P * m:P * (m + 1), :], xg_sb[m][:, :])

            # ---- phase 1a: xwT[j, r] = sum_c W_adj[c, j] x^T[c, r]  (fp8 DR)
            xwT8_sb = sb.tile([P, KT, R], F8, name="xwT8_sb", tag="xwT8_sb")
            for j in range(KT):
                pa = ps_a.tile([P, R], F32, name=f"psa{j}", tag="psa")
                for k in range(0, KT, 2):
                    nc.tensor.matmul(pa[:, :],
                                     adjW8_sb[:, k:k + 2, P * j:P * (j + 1)],
                                     xTs8_sb[:, k:k + 2, :],
                                     start=(k == 0), stop=(k == KT - 2),
                                     perf_mode=DR)
                nc.vector.tensor_copy(xwT8_sb[:, j, :], pa[:, :])

            # ---- phase 2: adjT tiles (fp8 DR), mask (bf16), deg ------------
            mask_sb = [sb.tile([P, R], BF16, name=f"mask{t}", tag=f"mask{t}") for t in range(NT)]
            pdeg = ps_deg.tile([1, R], F32, name="pdeg", tag="pdeg")
            for t in range(NT):
                pt = ps_adj.tile([P, R], F32, name=f"psadj{t}", tag="psadj")
                for k in range(0, KT, 2):
                    nc.tensor.matmul(pt[:, :],
                                     xT8_sb[:, k:k + 2, P * t:P * (t + 1)],
                                     xwT8_sb[:, k:k + 2, :],
                                     start=(k == 0), stop=(k == KT - 2),
                                     perf_mode=DR)
                nc.vector.tensor_scalar(mask_sb[t][:, :], pt[:, :], 0.0, None,
                                        mybir.AluOpType.not_equal)
                nc.tensor.matmul(pdeg[:, :], ones_col[:, :], mask_sb[t][:, :],
                                 start=(t == 0), stop=(t == NT - 1))

            deg_own = sb.tile([1, R], F32, name="deg_own", tag="deg_own")
            nc.vector.tensor_copy(deg_own[:, :], pdeg[:, :])
            # deg rides the payload as two bitcast bf16 rows
            nc.gpsimd.dma_start(yb_in[R:R + 2, :], deg_own[:, :].bitcast(BF16))

            # single AllGather: [xg ; deg] per rank
            nc.gpsimd.collective_compute(
                "AllGather", mybir.AluOpType.bypass, replica_groups=rg,
                ins=[yb_in.opt()], outs=[yb_out.opt()])

            # deg readbacks (gpsimd, ahead of the y DMAs): global from the
            # gathered payload, own-rows from the local payload buffer
            deg_glob = sb.tile([P, NT], F32, name="deg_glob", tag="deg_glob")
            deg_ownp = sb.tile([P, MT], F32, name="deg_ownp", tag="deg_ownp")
            ybof = yb_out.bitcast(F32).rearrange("(i r) c -> i (r c)", i=N_CORES)
            degof = ybof[:, R * C // 2:R * C // 2 + R]
            for i in range(N_CORES):
                nc.gpsimd.dma_start(
                    deg_glob[:, MT * i:MT * (i + 1)],
                    degof[i, :].rearrange("(t p) -> p t", p=P))
            ybif = yb_in.bitcast(F32).rearrange("r c -> (r c)")
            nc.gpsimd.dma_start(
                deg_ownp[:, :],
                ybif[R * C // 2:R * C // 2 + R].rearrange("(t p) -> p t", p=P))

            # y tiles: tile t lives at rank block t//4, quarter t%4
            y_mega = sb.tile([P, NT * C], BF16, name="y_mega", tag="y_mega")
            y_view = lambda t: y_mega[:, C * t:C * (t + 1)]
            for t in range(NT):
                eng = nc.sync if t % 2 == 0 else nc.scalar
                eng.dma_start(y_view(t),
                              yb_out[BR * (t // MT) + P * (t % MT):
                                     BR * (t // MT) + P * (t % MT + 1), :])

            # dis = deg^-1/2
            dis_glob = sb.tile([P, NT], F32, name="dis_glob", tag="dis_glob")
            dis_own = sb.tile([P, MT], F32, name="dis_own", tag="dis_own")
            nc.vector.reciprocal(dis_glob[:, :], deg_glob[:, :])
            nc.scalar.sqrt(dis_glob[:, :], dis_glob[:, :])
            nc.vector.reciprocal(dis_own[:, :], deg_ownp[:, :])
            nc.scalar.sqrt(dis_own[:, :], dis_own[:, :])
            # sqrt(deg) row-vector: cancels the dis_r row scaling for the bias.
            invdis_row = sb.tile([1, R], BF16, name="invdis_row", tag="invdis_row")
            nc.scalar.sqrt(invdis_row[:, :], deg_own[:, :])

            # ---- phase 3: y *= dis; out_rows = dis_r * (A @ y) + b ----------
            # per-tile scalings split across ACT and DVE so neither engine
            # serializes ahead of the final matmul
            for t in range(NT):
                if t % 2 == 0:
                    nc.scalar.mul(y_view(t), y_view(t), dis_glob[:, t:t + 1])
                else:
                    nc.vector.tensor_scalar(y_view(t), y_view(t),
                                            dis_glob[:, t:t + 1], None,
                                            mybir.AluOpType.mult)

            # m-outer: each PSUM bank accumulates a long 33-matmul chain
            # (bank-cycling per matmul triggers the HAM oscillation mode)
            for m in range(MT):
                pf = ps_fin.tile([P, C], F32, name=f"psf{m}", tag="psf")
                for t in range(NT):
                    nc.tensor.matmul(pf[:, :],
                                     mask_sb[t][:, P * m:P * (m + 1)],
                                     y_view(t),
                                     start=(t == 0), stop=False)
                # += sqrt(deg_r) (x) bias  — cancels against the dis_r scaling
                nc.tensor.matmul(pf[:, :],
                                 invdis_row[:, P * m:P * (m + 1)],
                                 bias_sb[:, :],
                                 start=False, stop=True)
                ot = sbo.tile([P, C], F32, name=f"outt{m}", tag="outt")
                nc.vector.tensor_scalar(ot[:, :], pf[:, :], dis_own[:, m:m + 1],
                                        None, mybir.AluOpType.mult)
                nc.sync.dma_start(out[P * m:P * (m + 1), :], ot[:, :])

    nc.compile()
    return nc


def _get_nc():
    if "nc" not in _cache:
        _cache["nc"] = _build()
    return _cache["nc"]


def _run(inputs, trace=False, trace_cores=None):
    x = np.asarray(inputs["x"], dtype=np.float32)
    adj_weight = np.asarray(inputs["adj_weight"], dtype=np.float32)
    gcn_weight = np.asarray(inputs["gcn_weight"], dtype=np.float32)
    gcn_bias = np.asarray(inputs["gcn_bias"], dtype=np.float32)

    xT = np.ascontiguousarray(x.T)                     # [C, N] f32
    xT8 = xT.astype(F8NP)
    adjW8 = adj_weight.astype(F8NP)
    gcnW = gcn_weight.astype(BF)
    bias_bf = gcn_bias.reshape(1, C).astype(BF)

    in_maps = []
    for i in range(N_CORES):
        sl = xT[:, R * i:R * (i + 1)]
        in_maps.append({
            "xT8": xT8,
            "xTs8": np.ascontiguousarray(xT8[:, R * i:R * (i + 1)]),
            "adjW8": adjW8,
            "xTs": np.ascontiguousarray(sl).astype(BF),
            "gcnW": gcnW,
            "bias": bias_bf,
        })

    nc = _get_nc()
    res = run_bass_kernel_spmd(nc, in_maps, core_ids=list(range(N_CORES)),
                               trace=trace, trace_cores=trace_cores)
    full = np.concatenate([res.results[i]["out"] for i in range(N_CORES)], axis=0)
    return full, res


def kernel(**inputs):
    full, _ = _run(inputs, trace=False)
    return full
